# revision 21
# baseline (speedup 1.0000x reference)
"""Trainium2 Bass kernel for nn_Attention_25967372271784.

Reference computation (note: q is computed but unused in the reference;
logits = K @ V^T * (head_dim * -0.5); softmax; out = attn @ V; y = out @ Wo.T).

Sharding: B(2) x H(16) = 32 (batch, head) pairs; 8 cores get 4 heads of one
batch each.  Projection weights are sliced per-core on host; the final
output projection is computed per-core as a partial sum over that core's
heads and reduced on host (y_full[b] = sum of the 4 cores of batch b).

Per-core device kernel (S=2048, D=64, E=1024, 4 heads):
  inputs:  xT [1024,2048] fp32 (x[b].T), WkT [1024,256] (=(-32*Wk[hsel]).T),
           WvT [1024,256], WoT [256,1024] (=Wo[:,hsel].T)
  kT_h = WkT_h-proj of xT     [64,2048]  (fp32r, scale folded into WkT)
  vT_h = WvT_h-proj of xT     [64,2048]  (fp32r)
  v_aug[t,65] per head: v natural + ones column (bf16)
  pass A: l_nat[s,t] tiles -> row max m[s] (DVE reduce, negate)
  m bounce via DRAM -> kTa_h row 64 (= -m[s]); vTa_h row 64 = ones
  pass B: l^T[t,s] - m[s] via K=65 matmul -> exp (ACT) -> pT bf16
  attn@v: u_aug^T[65,s] = v_aug^T . pT  (row 64 = Z[s])
  normalize: uN = u / Z (recip + partition_broadcast + mul)
  final: y_part[s,e] = sum_h uN_h @ WoT_h  (+ divide done above)
"""

import os
import sys

sys.path.insert(0, "/opt/trn_rl_repo")

# The kernel executes through jax's axon TRN backend; a JAX_PLATFORMS=cpu
# pin (common in reference harnesses) would hide the devices.
if os.environ.get("JAX_PLATFORMS", "").strip() == "cpu":
    del os.environ["JAX_PLATFORMS"]

import numpy as np

import concourse.bass as bass
import concourse.tile as tile
from concourse import mybir
from concourse import bass_utils

F32 = mybir.dt.float32
F32R = mybir.dt.float32r
BF16 = mybir.dt.bfloat16
F16 = mybir.dt.float16

B, S, E, H, D = 2, 2048, 1024, 16, 64
NH = 4          # heads per core
HD = NH * D     # 256 cols per core
N_CORES = 8
SCALE = D * -0.5  # -32.0

P = 128
NS_T = S // P          # 16 s-tiles of 128
EC = E // P            # 8 contraction chunks of 128
SC = 1024              # pass-B s-chunk width
NSC = S // SC          # 2 chunks
NT = S // P            # 16 t-tiles


def split_multiwaits(nc):
    """This container's walrus rejects >1 sync-wait on one instruction (seen
    on the Tile tail Drain).  Hoist extra waits onto preceding NoOps."""
    for f in nc.m.functions:
        for blk in f.blocks:
            insts = blk.instructions
            i = 0
            while i < len(insts):
                inst = insts[i]
                si = inst.sync_info
                if si is not None and len(si.on_wait) > 1:
                    waits = list(si.on_wait)
                    for w in waits[:-1]:
                        nop = mybir.InstNoOp(
                            name=nc.get_next_instruction_name(),
                            sync_info=mybir.SyncInfo(on_wait=[w], on_update=[]),
                            bass_nofuse=True,
                            engine=inst.engine,
                        )
                        nc.register_instruction(nop)
                        insts.insert(i, nop)
                        i += 1
                    si.on_wait = [waits[-1]]
                i += 1


def build_nc():
    nc = bass.Bass("TRN2", target_bir_lowering=False, debug=False, num_devices=1)

    xT1 = nc.dram_tensor("xT1", [E, S], BF16, kind="ExternalInput").ap()
    xT2 = nc.dram_tensor("xT2", [E, S], BF16, kind="ExternalInput").ap()
    WkT1 = nc.dram_tensor("WkT1", [E, NH * P], BF16, kind="ExternalInput").ap()
    WkT2 = nc.dram_tensor("WkT2", [E, NH * P], BF16, kind="ExternalInput").ap()
    WvT1 = nc.dram_tensor("WvT1", [E, NH * P], BF16, kind="ExternalInput").ap()
    WvT2 = nc.dram_tensor("WvT2", [E, NH * P], BF16, kind="ExternalInput").ap()
    WoT = nc.dram_tensor("WoT", [HD, E], F32, kind="ExternalInput").ap()
    y = nc.dram_tensor("y", [S, E], F32, kind="ExternalOutput").ap()
    import os as _os
    dbg = None
    if _os.environ.get("KDBG"):
        dbg = {
            "kTa0": nc.dram_tensor("dbg_kTa0", [P, S], F32, kind="ExternalOutput").ap(),
            "vTa0": nc.dram_tensor("dbg_vTa0", [P, S], F32, kind="ExternalOutput").ap(),
            "lt00": nc.dram_tensor("dbg_lt00", [P, SC], F32, kind="ExternalOutput").ap(),
        }
    # DRAM scratch for the per-head -m row bounce ([s] laid out flat)
    dram_m = [
        nc.dram_tensor(f"dram_m{h}", [S], F32R, kind="Internal").ap()
        for h in range(NH)
    ]
    dram_z = [
        nc.dram_tensor(f"dram_z{h}", [S], F32, kind="Internal").ap()
        for h in range(NH)
    ]

    with tile.TileContext(nc) as tc:
        build_kernel(tc, nc, (xT1, xT2), (WkT1, WkT2), (WvT1, WvT2), WoT, y, dram_m, dram_z, dbg)

    split_multiwaits(nc)
    return nc


def build_kernel(tc, nc, a_xT, a_WkT, a_WvT, WoT, y, dram_m, dram_z, dbg=None):
    from contextlib import ExitStack

    ctx = ExitStack()
    with ctx:
        persist = ctx.enter_context(tc.tile_pool(name="persist", bufs=1))

        # ---- phase 0: load inputs ----------------------------------------
        # x and the K/V projection weights arrive as bf16 (b1, b2) pairs
        # (b1 = bf16(val), b2 = bf16(val - b1)); three bf16 matmul terms
        # (b1*b1 + b1*b2 + b2*b1) give ~16-bit-precision projections with
        # exact bf16 products and fp32 PSUM accumulation.
        xT, WkT, WvT = a_xT, a_WkT, a_WvT
        xT_sb = [[], []]
        with tc.tile_pool(name="xT_pool", bufs=1) as xT_pool, \
             tc.tile_pool(name="wk_pool", bufs=1) as wk_pool:
            for i in range(2):
                for ec in range(EC):
                    t = xT_pool.tile([P, S], BF16, tag=f"xT{i}_{ec}", name=f"xT{i}_{ec}")
                    nc.sync.dma_start(out=t, in_=xT[i][ec * P:(ec + 1) * P, :])
                    xT_sb[i].append(t)
            wk_sb, wv_sb = [[], []], [[], []]
            for i in range(2):
                for ec in range(EC):
                    t = wk_pool.tile([P, NH * P], BF16, tag=f"wk{i}_{ec}", name=f"wk{i}_{ec}")
                    nc.sync.dma_start(out=t, in_=WkT[i][ec * P:(ec + 1) * P, :])
                    wk_sb[i].append(t)
                    t2 = wk_pool.tile([P, NH * P], BF16, tag=f"wv{i}_{ec}", name=f"wv{i}_{ec}")
                    nc.sync.dma_start(out=t2, in_=WvT[i][ec * P:(ec + 1) * P, :])
                    wv_sb[i].append(t2)

            # ---- phase 1: projections ------------------------------------
            kTa = [persist.tile([P, S], F32R, tag=f"kTa{h}", name=f"kTa{h}") for h in range(NH)]
            vTa = [persist.tile([P, S], F32R, tag=f"vTa{h}", name=f"vTa{h}") for h in range(NH)]
            vaug = [
                [persist.tile([P, D + 1], F16, tag=f"va{h}_{tt}", name=f"va{h}_{tt}") for tt in range(NT)]
                for h in range(NH)
            ]

            with tc.tile_pool(name="proj_ps", bufs=2, space="PSUM") as proj_ps:
                # kT / vT (transposed, scale folded into Wk).  Weight blocks
                # are host-prepped as [W_h | 0 | W_h[:, 0:63]], so PSUM rows
                # come out [k(64) ; 0 ; k(63 dup)].  fp32r rounds matmul
                # inputs to 12-bit mantissa: keep hi in rows 0-63 and the
                # residual (lo = k - fp32r(k)) in rows 65-127 so pass B's
                # K=128 contraction restores ~fp32 logit precision for free.
                # Row 64 is later overwritten with the aug row (-m / ones).
                for h in range(NH):
                    for wsb, dst in ((wk_sb, kTa[h]), (wv_sb, vTa[h])):
                        for nchunk in range(4):
                            ps = proj_ps.tile([P, 512], F32, tag="projps")
                            first = True
                            for ec in range(EC):
                                for wi, xi in ((0, 0), (0, 1), (1, 0)):
                                    nc.tensor.matmul(
                                        ps,
                                        wsb[wi][ec][:, h * P:(h + 1) * P],
                                        xT_sb[xi][ec][:, nchunk * 512:(nchunk + 1) * 512],
                                        start=first,
                                        stop=(ec == EC - 1 and (wi, xi) == (1, 0)),
                                    )
                                    first = False
                            sl = slice(nchunk * 512, (nchunk + 1) * 512)
                            nc.vector.tensor_copy(out=dst[0:D, sl], in_=ps[0:D, :])
                            nc.vector.tensor_copy(out=dst[D:P, sl], in_=ps[D:P, :])
                            nc.vector.tensor_sub(
                                dst[D:P, sl], ps[D:P, :], dst[D:P, sl]
                            )
                # ones aug row for vTa (memset can't write fp32r; cast-copy)
                ones_f32 = persist.tile([1, S], F32, tag="ones_f32", name="ones_f32")
                nc.vector.memset(ones_f32, 1.0)
                for h in range(NH):
                    nc.vector.tensor_copy(out=vTa[h][D:D + 1, :], in_=ones_f32)
                # v natural (all 4 heads at once): 16 t-tiles
                for tt in range(NT):
                    ps = proj_ps.tile([P, NH * P], F32, tag="vnatps")
                    first = True
                    for ec in range(EC):
                        for xi, wi in ((0, 0), (1, 0), (0, 1)):
                            nc.tensor.matmul(
                                ps,
                                xT_sb[xi][ec][:, tt * P:(tt + 1) * P],
                                wv_sb[wi][ec],
                                start=first,
                                stop=(ec == EC - 1 and (xi, wi) == (0, 1)),
                            )
                            first = False
                    for h in range(NH):
                        nc.vector.tensor_copy(
                            out=vaug[h][tt][:, 0:D], in_=ps[:, h * P:h * P + D]
                        )
                        nc.vector.memset(vaug[h][tt][:, D:D + 1], 1.0)
        # xT / WkT / WvT / staging SBUF released here.

        # uh / wo live from phase 2 to the end; their pool opens only after
        # phase 1 so they reuse the SBUF freed by xT/weight staging.
        late = ctx.enter_context(tc.tile_pool(name="late", bufs=1))
        wo_sb = []
        with tc.tile_pool(name="wost", bufs=2) as wost:
            for h in range(NH):
                st = wost.tile([D, E], F32, tag="wostage")
                nc.sync.dma_start(out=st, in_=WoT[h * D:(h + 1) * D, :])
                t = late.tile([D, E], F32R, tag=f"wo{h}", name=f"wo{h}")
                nc.vector.tensor_copy(out=t, in_=st)
                wo_sb.append(t)

        # ---- phase 2: attention per head ---------------------------------
        uh = [late.tile([P // 2 + 1, S], F32R, tag=f"uh{h}", name=f"uh{h}") for h in range(NH)]
        mstage = [persist.tile([P, NS_T], F32R, tag=f"ms{h}", name=f"ms{h}") for h in range(NH)]

        with tc.tile_pool(name="pa_ps", bufs=1, space="PSUM") as pa_ps, \
             tc.tile_pool(name="pb_ps", bufs=2, space="PSUM") as pb_ps, \
             tc.tile_pool(name="u_ps", bufs=2, space="PSUM") as u_ps, \
             tc.tile_pool(name="pt_pool", bufs=NT + 2) as pt_pool, \
             tc.tile_pool(name="sm_pool", bufs=4) as sm_pool, \
             tc.tile_pool(name="norm_pool", bufs=1) as norm_pool:
            for h in range(NH):
                # pass A: row maxes (negated) into mstage[h]
                for st_i in range(NS_T):
                    rmax = sm_pool.tile([P, 2], F32, tag="rmax")
                    for half in range(2):
                        ps = pa_ps.tile([P, 1024], F32, tag="paps")
                        for j in range(2):
                            tchunk = half * 1024 + j * 512
                            nc.tensor.matmul(
                                ps[:, j * 512:(j + 1) * 512],
                                kTa[h][0:D, st_i * P:(st_i + 1) * P],
                                vTa[h][0:D, tchunk:tchunk + 512],
                                start=True,
                                stop=True,
                            )
                        nc.vector.tensor_reduce(
                            out=rmax[:, half:half + 1],
                            in_=ps,
                            axis=mybir.AxisListType.X,
                            op=mybir.AluOpType.max,
                        )
                    nc.vector.tensor_reduce(
                        out=mstage[h][:, st_i:st_i + 1],
                        in_=rmax,
                        axis=mybir.AxisListType.X,
                        op=mybir.AluOpType.max,
                        negate=True,
                    )
                # bounce -m through DRAM into kTa[h] row 64 ([1, 2048])
                nc.sync.dma_start(
                    out=dram_m[h].rearrange("(i p) -> p i", p=P), in_=mstage[h]
                )
                nc.sync.dma_start(out=kTa[h][D:D + 1, :], in_=dram_m[h][None, :])

                # pass B + attn@v per s-chunk
                for sc_i in range(NSC):
                    s0 = sc_i * SC
                    pts = []
                    for tt in range(NT):
                        ps = pb_ps.tile([P, SC], F32, tag="pbps")
                        for j in range(SC // 512):
                            nc.tensor.matmul(
                                ps[:, j * 512:(j + 1) * 512],
                                vTa[h][:, tt * P:(tt + 1) * P],
                                kTa[h][:, s0 + j * 512:s0 + (j + 1) * 512],
                                start=True,
                                stop=True,
                            )
                        pt = pt_pool.tile([P, SC], F16, tag="pt")
                        nc.scalar.activation(
                            out=pt, in_=ps, func=mybir.ActivationFunctionType.Exp
                        )
                        pts.append(pt)
                        if dbg is not None and h == 0 and sc_i == 0 and tt == 0:
                            dbsb = norm_pool.tile([P, S], F32, tag="dbgt", name="dbsb")
                            nc.vector.tensor_copy(out=dbsb[:, 0:SC], in_=ps)
                            nc.sync.dma_start(out=dbg["lt00"], in_=dbsb[:, 0:SC])
                    # u_aug^T [65, SC] = sum_t v_aug[t,65].T @ pT[t, s]
                    for j in range(SC // 512):
                        ups = u_ps.tile([D + 1, 512], F32, tag="ups")
                        for tt in range(NT):
                            nc.tensor.matmul(
                                ups,
                                vaug[h][tt],
                                pts[tt][:, j * 512:(j + 1) * 512],
                                start=(tt == 0),
                                stop=(tt == NT - 1),
                            )
                        nc.vector.tensor_copy(
                            out=uh[h][:, s0 + j * 512:s0 + (j + 1) * 512], in_=ups
                        )

            if dbg is not None:
                dk = norm_pool.tile([P, S], F32, tag="dbgt", name="dk")
                nc.vector.tensor_copy(out=dk, in_=kTa[0])
                nc.sync.dma_start(out=dbg["kTa0"], in_=dk)
                dv = norm_pool.tile([P, S], F32, tag="dbgt", name="dv")
                nc.vector.tensor_copy(out=dv, in_=vTa[0])
                nc.sync.dma_start(out=dbg["vTa0"], in_=dv)

            # normalize: uN = u / Z  (Z = row 64 of uh)
            for h in range(NH):
                zrec = norm_pool.tile([1, S], F32, tag="zrec")
                zb = norm_pool.tile([D, S], F32, tag="zb")
                nc.vector.reciprocal(out=zrec, in_=uh[h][D:D + 1, :])
                nc.sync.dma_start(out=dram_z[h][None, :], in_=zrec)
                zrec_bcast = bass.AP(
                    tensor=dram_z[h].tensor, offset=dram_z[h].offset,
                    ap=[[0, D]] + list(dram_z[h].ap))
                nc.sync.dma_start(out=zb, in_=zrec_bcast)
                nc.vector.tensor_mul(uh[h][0:D, :], uh[h][0:D, :], zb)

        # ---- phase 3: final projection -----------------------------------
        with tc.tile_pool(name="y_ps", bufs=2, space="PSUM") as y_ps, \
             tc.tile_pool(name="y_sb", bufs=3) as y_sb_pool:
            for st_i in range(NS_T):
                for ecn in range(2):
                    ps = y_ps.tile([P, 512], F32, tag="yps")
                    for h in range(NH):
                        nc.tensor.matmul(
                            ps,
                            uh[h][0:D, st_i * P:(st_i + 1) * P],
                            wo_sb[h][:, ecn * 512:(ecn + 1) * 512],
                            start=(h == 0),
                            stop=(h == NH - 1),
                        )
                    ysb = y_sb_pool.tile([P, 512], F32, tag="ysb")
                    nc.vector.tensor_copy(out=ysb, in_=ps)
                    nc.sync.dma_start(
                        out=y[st_i * P:(st_i + 1) * P, ecn * 512:(ecn + 1) * 512],
                        in_=ysb,
                    )


_NC_CACHE = None


def _get_nc():
    global _NC_CACHE
    if _NC_CACHE is None:
        _NC_CACHE = build_nc()
    return _NC_CACHE


def make_in_maps(x, Wq, Wk, Wv, Wo):
    x = np.asarray(x, dtype=np.float32)
    Wk = np.asarray(Wk, dtype=np.float32)
    Wv = np.asarray(Wv, dtype=np.float32)
    Wo = np.asarray(Wo, dtype=np.float32)
    in_maps = []
    for c in range(N_CORES):
        b = c // 4
        h0 = (c % 4) * NH
        hsel = slice(h0 * D, (h0 + NH) * D)
        import ml_dtypes

        def bfpair(a):
            b1 = a.astype(ml_dtypes.bfloat16)
            b2 = (a - b1.astype(np.float32)).astype(ml_dtypes.bfloat16)
            return b1, b2

        def dupblocks(WT):  # [E, NH*D] -> [E, NH*128]: per head [W_h | 0 | W_h[:, :63]]
            blocks = []
            for h in range(NH):
                Wh = WT[:, h * D:(h + 1) * D]
                z = np.zeros((E, 1), WT.dtype)
                blocks.append(np.concatenate([Wh, z, Wh[:, 0:D - 1]], axis=1))
            return np.ascontiguousarray(np.concatenate(blocks, axis=1))

        x1, x2 = bfpair(np.ascontiguousarray(x[b].T))
        wk1, wk2 = bfpair((SCALE * Wk[hsel, :]).T)
        wv1, wv2 = bfpair(Wv[hsel, :].T)
        in_maps.append({
            "xT1": x1, "xT2": x2,
            "WkT1": dupblocks(wk1), "WkT2": dupblocks(wk2),
            "WvT1": dupblocks(wv1), "WvT2": dupblocks(wv2),
            "WoT": np.ascontiguousarray(Wo[:, hsel].T),
        })
    return in_maps


def kernel(x, Wq, Wk, Wv, Wo):
    nc = _get_nc()
    in_maps = make_in_maps(x, Wq, Wk, Wv, Wo)
    res = bass_utils.run_bass_kernel_spmd(nc, in_maps, core_ids=list(range(N_CORES)))
    out = np.zeros((B, S, E), dtype=np.float32)
    for c in range(N_CORES):
        out[c // 4] += res.results[c]["y"]
    return out


if __name__ == "__main__":
    rng = np.random.default_rng(0)
    x = rng.standard_normal((B, S, E), dtype=np.float32)
    std = 1.0 / np.sqrt(E)
    Wk = rng.standard_normal((E, E), dtype=np.float32) * std
    Wv = rng.standard_normal((E, E), dtype=np.float32) * std
    Wo = rng.standard_normal((E, E), dtype=np.float32) * std
    out = kernel(x, None, Wk, Wv, Wo)
    print("out", out.shape, out.dtype, float(np.abs(out).max()))


# revision 24
# speedup vs baseline: 1.0592x; 1.0592x over previous
"""Trainium2 Bass kernel for nn_Attention_25967372271784.

Reference computation (note: q is computed but unused in the reference;
logits = K @ V^T * (head_dim * -0.5); softmax; out = attn @ V; y = out @ Wo.T).

Sharding: B(2) x H(16) = 32 (batch, head) pairs; 8 cores get 4 heads of one
batch each.  Projection weights are sliced per-core on host; the final
output projection is computed per-core as a partial sum over that core's
heads and reduced on host (y_full[b] = sum of the 4 cores of batch b).

Per-core device kernel (S=2048, D=64, E=1024, 4 heads):
  inputs:  xT [1024,2048] fp32 (x[b].T), WkT [1024,256] (=(-32*Wk[hsel]).T),
           WvT [1024,256], WoT [256,1024] (=Wo[:,hsel].T)
  kT_h = WkT_h-proj of xT     [64,2048]  (fp32r, scale folded into WkT)
  vT_h = WvT_h-proj of xT     [64,2048]  (fp32r)
  v_aug[t,65] per head: v natural + ones column (bf16)
  pass A: l_nat[s,t] tiles -> row max m[s] (DVE reduce, negate)
  m bounce via DRAM -> kTa_h row 64 (= -m[s]); vTa_h row 64 = ones
  pass B: l^T[t,s] - m[s] via K=65 matmul -> exp (ACT) -> pT bf16
  attn@v: u_aug^T[65,s] = v_aug^T . pT  (row 64 = Z[s])
  normalize: uN = u / Z (recip + partition_broadcast + mul)
  final: y_part[s,e] = sum_h uN_h @ WoT_h  (+ divide done above)
"""

import os
import sys

sys.path.insert(0, "/opt/trn_rl_repo")

# The kernel executes through jax's axon TRN backend; a JAX_PLATFORMS=cpu
# pin (common in reference harnesses) would hide the devices.
if os.environ.get("JAX_PLATFORMS", "").strip() == "cpu":
    del os.environ["JAX_PLATFORMS"]

import numpy as np

import concourse.bass as bass
import concourse.tile as tile
from concourse import mybir
from concourse import bass_utils

F32 = mybir.dt.float32
F32R = mybir.dt.float32r
BF16 = mybir.dt.bfloat16
F16 = mybir.dt.float16

B, S, E, H, D = 2, 2048, 1024, 16, 64
NH = 4          # heads per core
HD = NH * D     # 256 cols per core
N_CORES = 8
SCALE = D * -0.5  # -32.0

P = 128
NS_T = S // P          # 16 s-tiles of 128
EC = E // P            # 8 contraction chunks of 128
SC = 1024              # pass-B s-chunk width
NSC = S // SC          # 2 chunks
NT = S // P            # 16 t-tiles


def split_multiwaits(nc):
    """This container's walrus rejects >1 sync-wait on one instruction (seen
    on the Tile tail Drain).  Hoist extra waits onto preceding NoOps."""
    for f in nc.m.functions:
        for blk in f.blocks:
            insts = blk.instructions
            i = 0
            while i < len(insts):
                inst = insts[i]
                si = inst.sync_info
                if si is not None and len(si.on_wait) > 1:
                    waits = list(si.on_wait)
                    for w in waits[:-1]:
                        nop = mybir.InstNoOp(
                            name=nc.get_next_instruction_name(),
                            sync_info=mybir.SyncInfo(on_wait=[w], on_update=[]),
                            bass_nofuse=True,
                            engine=inst.engine,
                        )
                        nc.register_instruction(nop)
                        insts.insert(i, nop)
                        i += 1
                    si.on_wait = [waits[-1]]
                i += 1


def build_nc():
    nc = bass.Bass("TRN2", target_bir_lowering=False, debug=False, num_devices=1)

    xT1 = nc.dram_tensor("xT1", [E, S], BF16, kind="ExternalInput").ap()
    xT2 = nc.dram_tensor("xT2", [E, S], BF16, kind="ExternalInput").ap()
    WkT1 = nc.dram_tensor("WkT1", [E, NH * P], BF16, kind="ExternalInput").ap()
    WkT2 = nc.dram_tensor("WkT2", [E, NH * P], BF16, kind="ExternalInput").ap()
    WvT1 = nc.dram_tensor("WvT1", [E, NH * P], BF16, kind="ExternalInput").ap()
    WvT2 = nc.dram_tensor("WvT2", [E, NH * P], BF16, kind="ExternalInput").ap()
    WoT = nc.dram_tensor("WoT", [HD, E], F32, kind="ExternalInput").ap()
    y = nc.dram_tensor("y", [S, E], F32, kind="ExternalOutput").ap()
    import os as _os
    dbg = None
    if _os.environ.get("KDBG"):
        dbg = {
            "kTa0": nc.dram_tensor("dbg_kTa0", [P, S], F32, kind="ExternalOutput").ap(),
            "vTa0": nc.dram_tensor("dbg_vTa0", [P, S], F32, kind="ExternalOutput").ap(),
            "lt00": nc.dram_tensor("dbg_lt00", [P, SC], F32, kind="ExternalOutput").ap(),
        }
    # DRAM scratch for the per-head -m row bounce ([s] laid out flat)
    dram_m = [
        nc.dram_tensor(f"dram_m{h}", [S], F32R, kind="Internal").ap()
        for h in range(NH)
    ]
    dram_z = [
        nc.dram_tensor(f"dram_z{h}", [S], F32, kind="Internal").ap()
        for h in range(NH)
    ]

    with tile.TileContext(nc) as tc:
        build_kernel(tc, nc, (xT1, xT2), (WkT1, WkT2), (WvT1, WvT2), WoT, y, dram_m, dram_z, dbg)

    split_multiwaits(nc)
    return nc


def build_kernel(tc, nc, a_xT, a_WkT, a_WvT, WoT, y, dram_m, dram_z, dbg=None):
    from contextlib import ExitStack

    ctx = ExitStack()
    with ctx:
        persist = ctx.enter_context(tc.tile_pool(name="persist", bufs=1))

        # ---- phase 0: load inputs ----------------------------------------
        # x and the K/V projection weights arrive as bf16 (b1, b2) pairs
        # (b1 = bf16(val), b2 = bf16(val - b1)); three bf16 matmul terms
        # (b1*b1 + b1*b2 + b2*b1) give ~16-bit-precision projections with
        # exact bf16 products and fp32 PSUM accumulation.
        xT, WkT, WvT = a_xT, a_WkT, a_WvT
        xT_sb = [[], []]
        with tc.tile_pool(name="xT_pool", bufs=1) as xT_pool, \
             tc.tile_pool(name="wk_pool", bufs=1) as wk_pool:
            for i in range(2):
                for ec in range(EC):
                    t = xT_pool.tile([P, S], BF16, tag=f"xT{i}_{ec}", name=f"xT{i}_{ec}")
                    nc.sync.dma_start(out=t, in_=xT[i][ec * P:(ec + 1) * P, :])
                    xT_sb[i].append(t)
            wk_sb, wv_sb = [[], []], [[], []]
            for i in range(2):
                for ec in range(EC):
                    t = wk_pool.tile([P, NH * P], BF16, tag=f"wk{i}_{ec}", name=f"wk{i}_{ec}")
                    nc.sync.dma_start(out=t, in_=WkT[i][ec * P:(ec + 1) * P, :])
                    wk_sb[i].append(t)
                    t2 = wk_pool.tile([P, NH * P], BF16, tag=f"wv{i}_{ec}", name=f"wv{i}_{ec}")
                    nc.sync.dma_start(out=t2, in_=WvT[i][ec * P:(ec + 1) * P, :])
                    wv_sb[i].append(t2)

            # ---- phase 1: projections ------------------------------------
            kTa = [persist.tile([P, S], F32R, tag=f"kTa{h}", name=f"kTa{h}") for h in range(NH)]
            vTa = [persist.tile([P, S], F32R, tag=f"vTa{h}", name=f"vTa{h}") for h in range(NH)]
            vaug = [
                [persist.tile([P, D + 1], F16, tag=f"va{h}_{tt}", name=f"va{h}_{tt}") for tt in range(NT)]
                for h in range(NH)
            ]

            with tc.tile_pool(name="proj_ps", bufs=2, space="PSUM") as proj_ps:
                # kT / vT (transposed, scale folded into Wk).  Weight blocks
                # are host-prepped as [W_h | 0 | W_h[:, 0:63]], so PSUM rows
                # come out [k(64) ; 0 ; k(63 dup)].  fp32r rounds matmul
                # inputs to 12-bit mantissa: keep hi in rows 0-63 and the
                # residual (lo = k - fp32r(k)) in rows 65-127 so pass B's
                # K=128 contraction restores ~fp32 logit precision for free.
                # Row 64 is later overwritten with the aug row (-m / ones).
                for h in range(NH):
                    for wsb, dst in ((wk_sb, kTa[h]), (wv_sb, vTa[h])):
                        for nchunk in range(4):
                            ps = proj_ps.tile([P, 512], F32, tag="projps")
                            first = True
                            for ec in range(EC):
                                for wi, xi in ((0, 0), (0, 1), (1, 0)):
                                    nc.tensor.matmul(
                                        ps,
                                        wsb[wi][ec][:, h * P:(h + 1) * P],
                                        xT_sb[xi][ec][:, nchunk * 512:(nchunk + 1) * 512],
                                        start=first,
                                        stop=(ec == EC - 1 and (wi, xi) == (1, 0)),
                                    )
                                    first = False
                            sl = slice(nchunk * 512, (nchunk + 1) * 512)
                            nc.vector.tensor_copy(out=dst[0:D, sl], in_=ps[0:D, :])
                            nc.vector.tensor_copy(out=dst[D:P, sl], in_=ps[D:P, :])
                            nc.vector.tensor_sub(
                                dst[D:P, sl], ps[D:P, :], dst[D:P, sl]
                            )
                # fp32r ones row: K=1 bias-matmul lhsT for pass B
                ones_f32 = persist.tile([1, S], F32, tag="ones_f32", name="ones_f32")
                nc.vector.memset(ones_f32, 1.0)
                # ones rows at partitions 0/32/64/96 (base must match the
                # per-head bias row's base in the K=1 matmul)
                ones_ra = persist.tile([65, P], F32R, tag="ones_ra", name="ones_ra")
                ones_rb = persist.tile([1, P], F32R, tag="ones_rb", name="ones_rb")
                for h in range(3):
                    nc.vector.tensor_copy(
                        out=ones_ra[32 * h:32 * h + 1, :], in_=ones_f32[:, 0:P]
                    )
                nc.vector.tensor_copy(out=ones_rb, in_=ones_f32[:, 0:P])
                ones_r = [ones_ra[0:1, :], ones_ra[32:33, :], ones_ra[64:65, :], ones_rb]
                # v natural (all 4 heads at once): 16 t-tiles
                for tt in range(NT):
                    ps = proj_ps.tile([P, NH * P], F32, tag="vnatps")
                    first = True
                    for ec in range(EC):
                        for xi, wi in ((0, 0), (1, 0), (0, 1)):
                            nc.tensor.matmul(
                                ps,
                                xT_sb[xi][ec][:, tt * P:(tt + 1) * P],
                                wv_sb[wi][ec],
                                start=first,
                                stop=(ec == EC - 1 and (xi, wi) == (0, 1)),
                            )
                            first = False
                    for h in range(NH):
                        nc.vector.tensor_copy(
                            out=vaug[h][tt][:, 0:D], in_=ps[:, h * P:h * P + D]
                        )
                        nc.vector.memset(vaug[h][tt][:, D:D + 1], 1.0)
        # xT / WkT / WvT / staging SBUF released here.

        # uh / wo live from phase 2 to the end; their pool opens only after
        # phase 1 so they reuse the SBUF freed by xT/weight staging.
        late = ctx.enter_context(tc.tile_pool(name="late", bufs=1))
        wo_sb = []
        with tc.tile_pool(name="wost", bufs=2) as wost:
            for h in range(NH):
                st = wost.tile([D, E], F32, tag="wostage")
                nc.sync.dma_start(out=st, in_=WoT[h * D:(h + 1) * D, :])
                t = late.tile([D, E], F32R, tag=f"wo{h}", name=f"wo{h}")
                nc.vector.tensor_copy(out=t, in_=st)
                wo_sb.append(t)

        # ---- phase 2: attention per head ---------------------------------
        uh = [late.tile([P // 2 + 1, S], F32R, tag=f"uh{h}", name=f"uh{h}") for h in range(NH)]
        mstage = [persist.tile([P, NS_T], F32R, tag=f"ms{h}", name=f"ms{h}") for h in range(NH)]
        # bias rows: 3 heads share one tile at partitions 0/32/64 (allowed
        # base partitions), head 3 gets its own
        mrow_a = late.tile([65, S], F32R, tag="mrow_a", name="mrow_a")
        mrow_b = late.tile([1, S], F32R, tag="mrow_b", name="mrow_b")
        mrow = [mrow_a[0:1, :], mrow_a[32:33, :], mrow_a[64:65, :], mrow_b]

        with tc.tile_pool(name="pa_ps", bufs=1, space="PSUM") as pa_ps, \
             tc.tile_pool(name="pb_ps", bufs=2, space="PSUM") as pb_ps, \
             tc.tile_pool(name="u_ps", bufs=2, space="PSUM") as u_ps, \
             tc.tile_pool(name="pt_pool", bufs=NT + 2) as pt_pool, \
             tc.tile_pool(name="sm_pool", bufs=4) as sm_pool, \
             tc.tile_pool(name="norm_pool", bufs=1) as norm_pool:
            for h in range(NH):
                # pass A: row maxes (negated) into mstage[h]
                for st_i in range(NS_T):
                    rmax = sm_pool.tile([P, 2], F32, tag="rmax")
                    for half in range(2):
                        ps = pa_ps.tile([P, 1024], F32, tag="paps")
                        for j in range(2):
                            tchunk = half * 1024 + j * 512
                            nc.tensor.matmul(
                                ps[:, j * 512:(j + 1) * 512],
                                kTa[h][0:D, st_i * P:(st_i + 1) * P],
                                vTa[h][0:D, tchunk:tchunk + 512],
                                start=True,
                                stop=True,
                            )
                        nc.vector.tensor_reduce(
                            out=rmax[:, half:half + 1],
                            in_=ps,
                            axis=mybir.AxisListType.X,
                            op=mybir.AluOpType.max,
                        )
                    nc.vector.tensor_reduce(
                        out=mstage[h][:, st_i:st_i + 1],
                        in_=rmax,
                        axis=mybir.AxisListType.X,
                        op=mybir.AluOpType.max,
                        negate=True,
                    )
                # bounce -m through DRAM into the [1, S] bias row
                nc.sync.dma_start(
                    out=dram_m[h].rearrange("(i p) -> p i", p=P), in_=mstage[h]
                )
                nc.sync.dma_start(out=mrow[h], in_=dram_m[h][None, :])

                # pass B + attn@v per s-chunk
                for sc_i in range(NSC):
                    s0 = sc_i * SC
                    pts = []
                    for tt in range(NT):
                        ps = pb_ps.tile([P, SC], F32, tag="pbps")
                        for j in range(SC // 512):
                            nc.tensor.matmul(
                                ps[:, j * 512:(j + 1) * 512],
                                vTa[h][:, tt * P:(tt + 1) * P],
                                kTa[h][:, s0 + j * 512:s0 + (j + 1) * 512],
                                start=True,
                                stop=False,
                            )
                            nc.tensor.matmul(
                                ps[:, j * 512:(j + 1) * 512],
                                ones_r[h],
                                mrow[h][:, s0 + j * 512:s0 + (j + 1) * 512],
                                start=False,
                                stop=True,
                            )
                        pt = pt_pool.tile([P, SC], F16, tag="pt")
                        nc.scalar.activation(
                            out=pt, in_=ps, func=mybir.ActivationFunctionType.Exp
                        )
                        pts.append(pt)
                        if dbg is not None and h == 0 and sc_i == 0 and tt == 0:
                            dbsb = norm_pool.tile([P, S], F32, tag="dbgt", name="dbsb")
                            nc.vector.tensor_copy(out=dbsb[:, 0:SC], in_=ps)
                            nc.sync.dma_start(out=dbg["lt00"], in_=dbsb[:, 0:SC])
                    # u_aug^T [65, SC] = sum_t v_aug[t,65].T @ pT[t, s]
                    for j in range(SC // 512):
                        ups = u_ps.tile([D + 1, 512], F32, tag="ups")
                        for tt in range(NT):
                            nc.tensor.matmul(
                                ups,
                                vaug[h][tt],
                                pts[tt][:, j * 512:(j + 1) * 512],
                                start=(tt == 0),
                                stop=(tt == NT - 1),
                            )
                        nc.vector.tensor_copy(
                            out=uh[h][:, s0 + j * 512:s0 + (j + 1) * 512], in_=ups
                        )

            if dbg is not None:
                dk = norm_pool.tile([P, S], F32, tag="dbgt", name="dk")
                nc.vector.tensor_copy(out=dk, in_=kTa[0])
                nc.sync.dma_start(out=dbg["kTa0"], in_=dk)
                dv = norm_pool.tile([P, S], F32, tag="dbgt", name="dv")
                nc.vector.tensor_copy(out=dv, in_=vTa[0])
                nc.sync.dma_start(out=dbg["vTa0"], in_=dv)

            # normalize: uN = u / Z  (Z = row 64 of uh)
            for h in range(NH):
                zrec = norm_pool.tile([1, S], F32, tag="zrec")
                zb = norm_pool.tile([D, S], F32, tag="zb")
                nc.vector.reciprocal(out=zrec, in_=uh[h][D:D + 1, :])
                nc.sync.dma_start(out=dram_z[h][None, :], in_=zrec)
                zrec_bcast = bass.AP(
                    tensor=dram_z[h].tensor, offset=dram_z[h].offset,
                    ap=[[0, D]] + list(dram_z[h].ap))
                nc.sync.dma_start(out=zb, in_=zrec_bcast)
                nc.vector.tensor_mul(uh[h][0:D, :], uh[h][0:D, :], zb)

        # ---- phase 3: final projection -----------------------------------
        with tc.tile_pool(name="y_ps", bufs=2, space="PSUM") as y_ps, \
             tc.tile_pool(name="y_sb", bufs=3) as y_sb_pool:
            for st_i in range(NS_T):
                for ecn in range(2):
                    ps = y_ps.tile([P, 512], F32, tag="yps")
                    for h in range(NH):
                        nc.tensor.matmul(
                            ps,
                            uh[h][0:D, st_i * P:(st_i + 1) * P],
                            wo_sb[h][:, ecn * 512:(ecn + 1) * 512],
                            start=(h == 0),
                            stop=(h == NH - 1),
                        )
                    ysb = y_sb_pool.tile([P, 512], F32, tag="ysb")
                    nc.vector.tensor_copy(out=ysb, in_=ps)
                    nc.sync.dma_start(
                        out=y[st_i * P:(st_i + 1) * P, ecn * 512:(ecn + 1) * 512],
                        in_=ysb,
                    )


_NC_CACHE = None


def _get_nc():
    global _NC_CACHE
    if _NC_CACHE is None:
        _NC_CACHE = build_nc()
    return _NC_CACHE


def make_in_maps(x, Wq, Wk, Wv, Wo):
    x = np.asarray(x, dtype=np.float32)
    Wk = np.asarray(Wk, dtype=np.float32)
    Wv = np.asarray(Wv, dtype=np.float32)
    Wo = np.asarray(Wo, dtype=np.float32)
    in_maps = []
    for c in range(N_CORES):
        b = c // 4
        h0 = (c % 4) * NH
        hsel = slice(h0 * D, (h0 + NH) * D)
        import ml_dtypes

        def bfpair(a):
            b1 = a.astype(ml_dtypes.bfloat16)
            b2 = (a - b1.astype(np.float32)).astype(ml_dtypes.bfloat16)
            return b1, b2

        def dupblocks(WT):  # [E, NH*D] -> [E, NH*128]: per head [W_h | W_h]
            blocks = []
            for h in range(NH):
                Wh = WT[:, h * D:(h + 1) * D]
                blocks.append(np.concatenate([Wh, Wh], axis=1))
            return np.ascontiguousarray(np.concatenate(blocks, axis=1))

        x1, x2 = bfpair(np.ascontiguousarray(x[b].T))
        wk1, wk2 = bfpair((SCALE * Wk[hsel, :]).T)
        wv1, wv2 = bfpair(Wv[hsel, :].T)
        in_maps.append({
            "xT1": x1, "xT2": x2,
            "WkT1": dupblocks(wk1), "WkT2": dupblocks(wk2),
            "WvT1": dupblocks(wv1), "WvT2": dupblocks(wv2),
            "WoT": np.ascontiguousarray(Wo[:, hsel].T),
        })
    return in_maps


def kernel(x, Wq, Wk, Wv, Wo):
    nc = _get_nc()
    in_maps = make_in_maps(x, Wq, Wk, Wv, Wo)
    res = bass_utils.run_bass_kernel_spmd(nc, in_maps, core_ids=list(range(N_CORES)))
    out = np.zeros((B, S, E), dtype=np.float32)
    for c in range(N_CORES):
        out[c // 4] += res.results[c]["y"]
    return out


if __name__ == "__main__":
    rng = np.random.default_rng(0)
    x = rng.standard_normal((B, S, E), dtype=np.float32)
    std = 1.0 / np.sqrt(E)
    Wk = rng.standard_normal((E, E), dtype=np.float32) * std
    Wv = rng.standard_normal((E, E), dtype=np.float32) * std
    Wo = rng.standard_normal((E, E), dtype=np.float32) * std
    out = kernel(x, None, Wk, Wv, Wo)
    print("out", out.shape, out.dtype, float(np.abs(out).max()))


# revision 28
# speedup vs baseline: 1.0774x; 1.0172x over previous
"""Trainium2 Bass kernel for nn_Attention_25967372271784.

Reference computation (note: q is computed but unused in the reference;
logits = K @ V^T * (head_dim * -0.5); softmax; out = attn @ V; y = out @ Wo.T).

Sharding: B(2) x H(16) = 32 (batch, head) pairs; 8 cores get 4 heads of one
batch each.  Projection weights are sliced per-core on host; the final
output projection is computed per-core as a partial sum over that core's
heads and reduced on host (y_full[b] = sum of the 4 cores of batch b).

Per-core device kernel (S=2048, D=64, E=1024, 4 heads):
  inputs:  xT [1024,2048] fp32 (x[b].T), WkT [1024,256] (=(-32*Wk[hsel]).T),
           WvT [1024,256], WoT [256,1024] (=Wo[:,hsel].T)
  kT_h = WkT_h-proj of xT     [64,2048]  (fp32r, scale folded into WkT)
  vT_h = WvT_h-proj of xT     [64,2048]  (fp32r)
  v_aug[t,65] per head: v natural + ones column (bf16)
  pass A: l_nat[s,t] tiles -> row max m[s] (DVE reduce, negate)
  m bounce via DRAM -> kTa_h row 64 (= -m[s]); vTa_h row 64 = ones
  pass B: l^T[t,s] - m[s] via K=65 matmul -> exp (ACT) -> pT bf16
  attn@v: u_aug^T[65,s] = v_aug^T . pT  (row 64 = Z[s])
  normalize: uN = u / Z (recip + partition_broadcast + mul)
  final: y_part[s,e] = sum_h uN_h @ WoT_h  (+ divide done above)
"""

import os
import sys

sys.path.insert(0, "/opt/trn_rl_repo")

# The kernel executes through jax's axon TRN backend; a JAX_PLATFORMS=cpu
# pin (common in reference harnesses) would hide the devices.
if os.environ.get("JAX_PLATFORMS", "").strip() == "cpu":
    del os.environ["JAX_PLATFORMS"]

import numpy as np

import concourse.bass as bass
import concourse.tile as tile
from concourse import mybir
from concourse import bass_utils

F32 = mybir.dt.float32
F32R = mybir.dt.float32r
BF16 = mybir.dt.bfloat16
F16 = mybir.dt.float16

B, S, E, H, D = 2, 2048, 1024, 16, 64
NH = 4          # heads per core
HD = NH * D     # 256 cols per core
N_CORES = 8
SCALE = D * -0.5  # -32.0

P = 128
NS_T = S // P          # 16 s-tiles of 128
EC = E // P            # 8 contraction chunks of 128
SC = 1024              # pass-B s-chunk width
NSC = S // SC          # 2 chunks
NT = S // P            # 16 t-tiles


def split_multiwaits(nc):
    """This container's walrus rejects >1 sync-wait on one instruction (seen
    on the Tile tail Drain).  Hoist extra waits onto preceding NoOps."""
    for f in nc.m.functions:
        for blk in f.blocks:
            insts = blk.instructions
            i = 0
            while i < len(insts):
                inst = insts[i]
                si = inst.sync_info
                if si is not None and len(si.on_wait) > 1:
                    waits = list(si.on_wait)
                    for w in waits[:-1]:
                        nop = mybir.InstNoOp(
                            name=nc.get_next_instruction_name(),
                            sync_info=mybir.SyncInfo(on_wait=[w], on_update=[]),
                            bass_nofuse=True,
                            engine=inst.engine,
                        )
                        nc.register_instruction(nop)
                        insts.insert(i, nop)
                        i += 1
                    si.on_wait = [waits[-1]]
                i += 1


def build_nc():
    nc = bass.Bass("TRN2", target_bir_lowering=False, debug=False, num_devices=1)

    xT1 = nc.dram_tensor("xT1", [E, S], BF16, kind="ExternalInput").ap()
    xT2 = nc.dram_tensor("xT2", [E, S], BF16, kind="ExternalInput").ap()
    WkT1 = nc.dram_tensor("WkT1", [E, NH * P], BF16, kind="ExternalInput").ap()
    WkT2 = nc.dram_tensor("WkT2", [E, NH * P], BF16, kind="ExternalInput").ap()
    WvT1 = nc.dram_tensor("WvT1", [E, NH * P], BF16, kind="ExternalInput").ap()
    WvT2 = nc.dram_tensor("WvT2", [E, NH * P], BF16, kind="ExternalInput").ap()
    WoT = nc.dram_tensor("WoT", [HD, E], F32, kind="ExternalInput").ap()
    y = nc.dram_tensor("y", [S, E], F32, kind="ExternalOutput").ap()
    import os as _os
    dbg = None
    if _os.environ.get("KDBG"):
        dbg = {
            "kTa0": nc.dram_tensor("dbg_kTa0", [P, S], F32, kind="ExternalOutput").ap(),
            "vTa0": nc.dram_tensor("dbg_vTa0", [P, S], F32, kind="ExternalOutput").ap(),
            "lt00": nc.dram_tensor("dbg_lt00", [P, SC], F32, kind="ExternalOutput").ap(),
            "uh0": nc.dram_tensor("dbg_uh0", [P, S], F32, kind="ExternalOutput").ap(),
            "pt1": nc.dram_tensor("dbg_pt1", [S, SC], F32, kind="ExternalOutput").ap(),
            "lt1": nc.dram_tensor("dbg_lt1", [S, SC], F32, kind="ExternalOutput").ap(),
        }
    # DRAM scratch for the per-head -m row bounce ([s] laid out flat)
    dram_m = [
        nc.dram_tensor(f"dram_m{h}", [S], F32R, kind="Internal").ap()
        for h in range(NH)
    ]
    dram_z = [
        nc.dram_tensor(f"dram_z{h}", [S], F32, kind="Internal").ap()
        for h in range(NH)
    ]

    with tile.TileContext(nc) as tc:
        build_kernel(tc, nc, (xT1, xT2), (WkT1, WkT2), (WvT1, WvT2), WoT, y, dram_m, dram_z, dbg)

    split_multiwaits(nc)
    return nc


def build_kernel(tc, nc, a_xT, a_WkT, a_WvT, WoT, y, dram_m, dram_z, dbg=None):
    """Head-pair tile layout (pair p = heads 2p, 2p+1), all [128, S] fp32r:

      T1_p = [vh0 ; vl0]     T2_p = [kh0 ; kh0]     T3_p = [kl0 ; kl1]
      T4_p = [vl1 ; vh1]     T5_p = [kh1 ; kh1]

    where kh/vh = fp32r(k or v) (12-bit mantissa) and kl/vl the residual.
    Matmul rows pair positionally and operand base partitions must match,
    so per pass-B tile:
      h even: [T1 x T2](K=128) -> vh*kh + vl*kh ;  [T1 x T3](rows 0:64) -> vh*kl
      h odd:  [T4 x T5](K=128) -> vh*kh + vl*kh ;  [T4 x T3](rows 64:128) -> vh*kl
    which with fp32 PSUM accumulation restores ~fp32 logits (only vl*kl,
    ~1e-8 relative, is dropped).  The -m softmax bias is a separate K=1
    ones-row matmul into the same PSUM accumulation group.
    """
    from contextlib import ExitStack

    ctx = ExitStack()
    with ctx:
        persist = ctx.enter_context(tc.tile_pool(name="persist", bufs=1))

        # ---- phase 0: load inputs (bf16 b1/b2 pairs; see make_in_maps) ----
        xT, WkT, WvT = a_xT, a_WkT, a_WvT
        xT_sb = [[], []]
        with tc.tile_pool(name="xT_pool", bufs=1) as xT_pool, \
             tc.tile_pool(name="wk_pool", bufs=1) as wk_pool:
            for i in range(2):
                for ec in range(EC):
                    t = xT_pool.tile([P, S], BF16, tag=f"xT{i}_{ec}", name=f"xT{i}_{ec}")
                    nc.sync.dma_start(out=t, in_=xT[i][ec * P:(ec + 1) * P, :])
                    xT_sb[i].append(t)
            wk_sb, wv_sb = [[], []], [[], []]
            for i in range(2):
                for ec in range(EC):
                    t = wk_pool.tile([P, NH * P], BF16, tag=f"wk{i}_{ec}", name=f"wk{i}_{ec}")
                    nc.sync.dma_start(out=t, in_=WkT[i][ec * P:(ec + 1) * P, :])
                    wk_sb[i].append(t)
                    t2 = wk_pool.tile([P, NH * P], BF16, tag=f"wv{i}_{ec}", name=f"wv{i}_{ec}")
                    nc.sync.dma_start(out=t2, in_=WvT[i][ec * P:(ec + 1) * P, :])
                    wv_sb[i].append(t2)

            # ---- phase 1: projections ------------------------------------
            NP = NH // 2
            T1 = [persist.tile([P, S], F32R, tag=f"T1_{p}", name=f"T1_{p}") for p in range(NP)]
            T2 = [persist.tile([P, S], F32R, tag=f"T2_{p}", name=f"T2_{p}") for p in range(NP)]
            T3 = [persist.tile([P, S], F32R, tag=f"T3_{p}", name=f"T3_{p}") for p in range(NP)]
            T4 = [persist.tile([P, S], F32R, tag=f"T4_{p}", name=f"T4_{p}") for p in range(NP)]
            T5 = [persist.tile([P, S], F32R, tag=f"T5_{p}", name=f"T5_{p}") for p in range(NP)]
            vaug = [
                [persist.tile([P, D + 1], F16, tag=f"va{h}_{tt}", name=f"va{h}_{tt}") for tt in range(NT)]
                for h in range(NH)
            ]

            with tc.tile_pool(name="proj_ps", bufs=2, space="PSUM") as proj_ps:
                # Weight blocks are host-prepped as [W_h | W_h] so PSUM rows
                # come out [val(64) ; val(64)] (the dup makes both partition
                # halves addressable at aligned bases).
                for h in range(NH):
                    p, odd = divmod(h, 2)
                    for wsb, kind in ((wk_sb, "k"), (wv_sb, "v")):
                        for nchunk in range(4):
                            ps = proj_ps.tile([P, 512], F32, tag="projps")
                            first = True
                            for ec in range(EC):
                                for wi, xi in ((0, 0), (0, 1), (1, 0)):
                                    nc.tensor.matmul(
                                        ps,
                                        wsb[wi][ec][:, h * P:(h + 1) * P],
                                        xT_sb[xi][ec][:, nchunk * 512:(nchunk + 1) * 512],
                                        start=first,
                                        stop=(ec == EC - 1 and (wi, xi) == (1, 0)),
                                    )
                                    first = False
                            sl = slice(nchunk * 512, (nchunk + 1) * 512)
                            lo_half = slice(D, P)
                            hi_half = slice(0, D)
                            if kind == "k" and not odd:
                                nc.vector.tensor_copy(out=T2[p][hi_half, sl], in_=ps[hi_half, :])
                                nc.vector.tensor_copy(out=T2[p][lo_half, sl], in_=ps[lo_half, :])
                                nc.vector.tensor_copy(out=T3[p][hi_half, sl], in_=ps[hi_half, :])
                                nc.vector.tensor_sub(T3[p][hi_half, sl], ps[hi_half, :], T3[p][hi_half, sl])
                            elif kind == "k" and odd:
                                nc.vector.tensor_copy(out=T5[p][hi_half, sl], in_=ps[hi_half, :])
                                nc.vector.tensor_copy(out=T5[p][lo_half, sl], in_=ps[lo_half, :])
                                nc.vector.tensor_copy(out=T3[p][lo_half, sl], in_=ps[lo_half, :])
                                nc.vector.tensor_sub(T3[p][lo_half, sl], ps[lo_half, :], T3[p][lo_half, sl])
                            elif kind == "v" and not odd:
                                nc.vector.tensor_copy(out=T1[p][hi_half, sl], in_=ps[hi_half, :])
                                nc.vector.tensor_copy(out=T1[p][lo_half, sl], in_=ps[lo_half, :])
                                nc.vector.tensor_sub(T1[p][lo_half, sl], ps[lo_half, :], T1[p][lo_half, sl])
                            else:
                                nc.vector.tensor_copy(out=T4[p][lo_half, sl], in_=ps[lo_half, :])
                                nc.vector.tensor_copy(out=T4[p][hi_half, sl], in_=ps[hi_half, :])
                                nc.vector.tensor_sub(T4[p][hi_half, sl], ps[hi_half, :], T4[p][hi_half, sl])
                # fp32r ones rows for the K=1 bias matmuls, at legal bases
                ones_f32 = persist.tile([1, P], F32, tag="ones_f32", name="ones_f32")
                nc.vector.memset(ones_f32, 1.0)
                ones_ra = persist.tile([65, P], F32R, tag="ones_ra", name="ones_ra")
                ones_rb = persist.tile([1, P], F32R, tag="ones_rb", name="ones_rb")
                for h in range(3):
                    nc.vector.tensor_copy(out=ones_ra[32 * h:32 * h + 1, :], in_=ones_f32)
                nc.vector.tensor_copy(out=ones_rb, in_=ones_f32)
                ones_r = [ones_ra[0:1, :], ones_ra[32:33, :], ones_ra[64:65, :], ones_rb]
                # v natural (all 4 heads at once): 16 t-tiles
                for tt in range(NT):
                    ps = proj_ps.tile([P, NH * P], F32, tag="vnatps")
                    first = True
                    for ec in range(EC):
                        for xi, wi in ((0, 0), (1, 0), (0, 1)):
                            nc.tensor.matmul(
                                ps,
                                xT_sb[xi][ec][:, tt * P:(tt + 1) * P],
                                wv_sb[wi][ec],
                                start=first,
                                stop=(ec == EC - 1 and (xi, wi) == (0, 1)),
                            )
                            first = False
                    for h in range(NH):
                        nc.vector.tensor_copy(
                            out=vaug[h][tt][:, 0:D], in_=ps[:, h * P:h * P + D]
                        )
                        nc.vector.memset(vaug[h][tt][:, D:D + 1], 1.0)
        # xT / weight staging SBUF released here.

        # per-head operand views for pass A / pass B
        def pa_ops(h):  # (lhsT source rows = k hi, rhs source rows = v hi)
            p, odd = divmod(h, 2)
            if not odd:
                return T2[p][0:D, :], T1[p][0:D, :]
            return T5[p][D:P, :], T4[p][D:P, :]

        def pb_main(h):  # K=128 tiles (lhsT, rhs)
            p, odd = divmod(h, 2)
            return (T4[p], T5[p]) if odd else (T1[p], T2[p])

        def pb_cross(h):  # K=64 row-slices (lhsT, rhs)
            p, odd = divmod(h, 2)
            if not odd:
                return T1[p][0:D, :], T3[p][0:D, :]
            return T4[p][D:P, :], T3[p][D:P, :]

        late = ctx.enter_context(tc.tile_pool(name="late", bufs=1))
        uh = [late.tile([P // 2 + 1, S], F32R, tag=f"uh{h}", name=f"uh{h}") for h in range(NH)]
        mstage = [persist.tile([P, NS_T], F32R, tag=f"ms{h}", name=f"ms{h}") for h in range(NH)]
        # bias rows: 3 heads share one tile at partitions 0/32/64, head 3 solo
        mrow_a = late.tile([65, S], F32R, tag="mrow_a", name="mrow_a")
        mrow_b = late.tile([1, S], F32R, tag="mrow_b", name="mrow_b")
        mrow = [mrow_a[0:1, :], mrow_a[32:33, :], mrow_a[64:65, :], mrow_b]

        # ---- phase 2: attention per head ---------------------------------
        with tc.tile_pool(name="pa_ps", bufs=1, space="PSUM") as pa_ps, \
             tc.tile_pool(name="pb_ps", bufs=2, space="PSUM") as pb_ps, \
             tc.tile_pool(name="u_ps", bufs=2, space="PSUM") as u_ps, \
             tc.tile_pool(name="pt_pool", bufs=NT + 2) as pt_pool, \
             tc.tile_pool(name="sm_pool", bufs=4) as sm_pool:
            for h in range(NH):
                kx, vx = pa_ops(h)
                # pass A: row maxes (negated) into mstage[h]
                for st_i in range(NS_T):
                    rmax = sm_pool.tile([P, 2], F32, tag="rmax")
                    for half in range(2):
                        ps = pa_ps.tile([P, 1024], F32, tag="paps")
                        for j in range(2):
                            tchunk = half * 1024 + j * 512
                            nc.tensor.matmul(
                                ps[:, j * 512:(j + 1) * 512],
                                kx[:, st_i * P:(st_i + 1) * P],
                                vx[:, tchunk:tchunk + 512],
                                start=True,
                                stop=True,
                            )
                        nc.vector.tensor_reduce(
                            out=rmax[:, half:half + 1],
                            in_=ps,
                            axis=mybir.AxisListType.X,
                            op=mybir.AluOpType.max,
                        )
                    nc.vector.tensor_reduce(
                        out=mstage[h][:, st_i:st_i + 1],
                        in_=rmax,
                        axis=mybir.AxisListType.X,
                        op=mybir.AluOpType.max,
                        negate=True,
                    )
                # bounce -m through DRAM into the [1, S] bias row
                nc.sync.dma_start(
                    out=dram_m[h].rearrange("(i p) -> p i", p=P), in_=mstage[h]
                )
                nc.sync.dma_start(out=mrow[h], in_=dram_m[h][None, :])

                # pass B + attn@v per s-chunk
                ma, mb = pb_main(h)
                ca, cb = pb_cross(h)
                for sc_i in range(NSC):
                    s0 = sc_i * SC
                    pts = []
                    for tt in range(NT):
                        ps = pb_ps.tile([P, SC], F32, tag="pbps")
                        for j in range(SC // 512):
                            ssl = slice(s0 + j * 512, s0 + (j + 1) * 512)
                            osl = slice(j * 512, (j + 1) * 512)
                            tsl = slice(tt * P, (tt + 1) * P)
                            nc.tensor.matmul(
                                ps[:, osl], ma[:, tsl], mb[:, ssl],
                                start=True, stop=False,
                            )
                            nc.tensor.matmul(
                                ps[:, osl], ca[:, tsl], cb[:, ssl],
                                start=False, stop=False,
                            )
                            nc.tensor.matmul(
                                ps[:, osl], ones_r[h], mrow[h][:, ssl],
                                start=False, stop=True,
                            )
                        pt = pt_pool.tile([P, SC], F16, tag="pt")
                        nc.scalar.activation(
                            out=pt, in_=ps, func=mybir.ActivationFunctionType.Exp
                        )
                        pts.append(pt)
                        if dbg is not None and h == 0 and sc_i == 1:
                            dpt = sm_pool.tile([P, SC], F32, tag="dbgt", name=f"dpt{tt}", bufs=1)
                            nc.vector.tensor_copy(out=dpt, in_=pt)
                            nc.sync.dma_start(
                                out=dbg["pt1"][tt * P:(tt + 1) * P, :], in_=dpt
                            )
                    # u_aug^T [65, SC] = sum_t v_aug[t,65].T @ pT[t, s]
                    for j in range(SC // 512):
                        ups = u_ps.tile([D + 1, 512], F32, tag="ups")
                        for tt in range(NT):
                            nc.tensor.matmul(
                                ups,
                                vaug[h][tt],
                                pts[tt][:, j * 512:(j + 1) * 512],
                                start=(tt == 0),
                                stop=(tt == NT - 1),
                            )
                        nc.vector.tensor_copy(
                            out=uh[h][:, s0 + j * 512:s0 + (j + 1) * 512], in_=ups
                        )

        # ---- normalize: uN = u / Z  (Z = row 64 of uh) --------------------
        with tc.tile_pool(name="norm_pool", bufs=1) as norm_pool:
            for h in range(NH):
                zrec = norm_pool.tile([1, S], F32, tag="zrec")
                zb = norm_pool.tile([D, S], F32, tag="zb")
                nc.vector.reciprocal(out=zrec, in_=uh[h][D:D + 1, :])
                nc.sync.dma_start(out=dram_z[h][None, :], in_=zrec)
                zrec_bcast = bass.AP(
                    tensor=dram_z[h].tensor, offset=dram_z[h].offset,
                    ap=[[0, D]] + list(dram_z[h].ap))
                nc.sync.dma_start(out=zb, in_=zrec_bcast)
                nc.vector.tensor_mul(uh[h][0:D, :], uh[h][0:D, :], zb)

        # ---- phase 3: final projection -----------------------------------
        wo_sb = []
        with tc.tile_pool(name="wost", bufs=2) as wost:
            for h in range(NH):
                st = wost.tile([D, E], F32, tag="wostage")
                nc.sync.dma_start(out=st, in_=WoT[h * D:(h + 1) * D, :])
                t = late.tile([D, E], F32R, tag=f"wo{h}", name=f"wo{h}")
                nc.vector.tensor_copy(out=t, in_=st)
                wo_sb.append(t)
        with tc.tile_pool(name="y_ps", bufs=2, space="PSUM") as y_ps, \
             tc.tile_pool(name="y_sb", bufs=3) as y_sb_pool:
            for st_i in range(NS_T):
                for ecn in range(2):
                    ps = y_ps.tile([P, 512], F32, tag="yps")
                    for h in range(NH):
                        nc.tensor.matmul(
                            ps,
                            uh[h][0:D, st_i * P:(st_i + 1) * P],
                            wo_sb[h][:, ecn * 512:(ecn + 1) * 512],
                            start=(h == 0),
                            stop=(h == NH - 1),
                        )
                    ysb = y_sb_pool.tile([P, 512], F32, tag="ysb")
                    nc.vector.tensor_copy(out=ysb, in_=ps)
                    nc.sync.dma_start(
                        out=y[st_i * P:(st_i + 1) * P, ecn * 512:(ecn + 1) * 512],
                        in_=ysb,
                    )


_NC_CACHE = None


def _get_nc():
    global _NC_CACHE
    if _NC_CACHE is None:
        _NC_CACHE = build_nc()
    return _NC_CACHE


def make_in_maps(x, Wq, Wk, Wv, Wo):
    x = np.asarray(x, dtype=np.float32)
    Wk = np.asarray(Wk, dtype=np.float32)
    Wv = np.asarray(Wv, dtype=np.float32)
    Wo = np.asarray(Wo, dtype=np.float32)
    in_maps = []
    for c in range(N_CORES):
        b = c // 4
        h0 = (c % 4) * NH
        hsel = slice(h0 * D, (h0 + NH) * D)
        import ml_dtypes

        def bfpair(a):
            b1 = a.astype(ml_dtypes.bfloat16)
            b2 = (a - b1.astype(np.float32)).astype(ml_dtypes.bfloat16)
            return b1, b2

        def dupblocks(WT):  # [E, NH*D] -> [E, NH*128]: per head [W_h | W_h]
            blocks = []
            for h in range(NH):
                Wh = WT[:, h * D:(h + 1) * D]
                blocks.append(np.concatenate([Wh, Wh], axis=1))
            return np.ascontiguousarray(np.concatenate(blocks, axis=1))

        x1, x2 = bfpair(np.ascontiguousarray(x[b].T))
        wk1, wk2 = bfpair((SCALE * Wk[hsel, :]).T)
        wv1, wv2 = bfpair(Wv[hsel, :].T)
        in_maps.append({
            "xT1": x1, "xT2": x2,
            "WkT1": dupblocks(wk1), "WkT2": dupblocks(wk2),
            "WvT1": dupblocks(wv1), "WvT2": dupblocks(wv2),
            "WoT": np.ascontiguousarray(Wo[:, hsel].T),
        })
    return in_maps


def kernel(x, Wq, Wk, Wv, Wo):
    nc = _get_nc()
    in_maps = make_in_maps(x, Wq, Wk, Wv, Wo)
    res = bass_utils.run_bass_kernel_spmd(nc, in_maps, core_ids=list(range(N_CORES)))
    out = np.zeros((B, S, E), dtype=np.float32)
    for c in range(N_CORES):
        out[c // 4] += res.results[c]["y"]
    return out


if __name__ == "__main__":
    rng = np.random.default_rng(0)
    x = rng.standard_normal((B, S, E), dtype=np.float32)
    std = 1.0 / np.sqrt(E)
    Wk = rng.standard_normal((E, E), dtype=np.float32) * std
    Wv = rng.standard_normal((E, E), dtype=np.float32) * std
    Wo = rng.standard_normal((E, E), dtype=np.float32) * std
    out = kernel(x, None, Wk, Wv, Wo)
    print("out", out.shape, out.dtype, float(np.abs(out).max()))
